# revision 31
# baseline (speedup 1.0000x reference)
"""Trainium2 Bass kernel for nn_CylindricalPointEncoder (segment_reduce).

Pipeline (8 NeuronCores, SPMD):
  host: voxel-key counting sort; voxels split into <=64-point chunks,
        padded (duplicate points) to multiples of 8 slots; greedy-packed
        into 512-point tiles (max 16 chunks/tile); per-core slabs of
        transposed features.
  P1 (host, tiny): moments of raw feats -> BN0 fold + analytic BN1 stats.
  P2 (device): x2 = relu(f @ A1 + d1) ; masked moments [64, 65] -> host
        folds BN2.
  P3 (device): x3 moments [128, 129] -> host folds BN3.
  P4 (device): full MLP -> h4 [256 x 512] per tile -> 8-slot bin max
        (DVE reduce) -> one wide indirect_copy gather of clamped bin
        indices -> per-voxel max -> compact output columns.
  host: reassemble [V, 256] (+ folded bias), emit unq_coords.

BN folding: y = BN(x) @ W + b == x @ (a*W) + ((be - m*a) @ W + b) with
a = g/sqrt(v+eps); stats of pre-activations derived analytically from
moments of the (relu'd) previous layer, so only two device stat passes
are needed. relu biases are folded into the next layer's additive term
(x' = max(h, -d); x = x' + d) where convenient.
"""

import math
import numpy as np

# ---------------------------------------------------------------------------
# compat: the walrus build in this container rejects InstDrain carrying sem
# waits/updates on PE/ACT/DVE/SP ("Too many sync wait commands"). Replace the
# tile barrier + tail drain with split nop/sem_inc/wait_ge sequences.
# ---------------------------------------------------------------------------
import bass_rust
import concourse.bass as bass
import concourse.mybir as mybir
import concourse.tile as tile
from concourse.vector_clock import ScopedClock
from concourse.bass_utils import run_bass_kernel_spmd
from concourse.masks import make_identity


def _ensure_barrier_sem(nc):
    if getattr(nc, "_compat_aeb_sem", None) is None:
        nc._compat_aeb_sem = nc.alloc_semaphore("compat_aeb_sem")
        nc._compat_aeb_count = 0
    return nc._compat_aeb_sem


def _patched_multi_engine_barrier(self, engines):
    sem = _ensure_barrier_sem(self)
    target = self._compat_aeb_count + len(engines)
    for eng_t in engines:
        eng = self.engines[eng_t]
        eng.drain(fusable=False)
        eng.sem_inc(sem, 1)
    for eng_t in engines:
        self.engines[eng_t].wait_ge(sem, target)
    self._compat_aeb_count = target


def _patched_drain_and_barrier(self, tick_clock, wait_clock):
    nc = self.nc
    probe = nc.sync.nop(nofuse=True)
    wait_clock.add_sem_waits(probe.ins, ScopedClock({None: tick_clock.global_clock}))
    si = probe.ins.sync_info
    waits = list(si.on_wait) if si is not None else []
    probe.ins.sync_info = bass_rust.SyncInfo(on_wait=[], on_update=[])
    for w in waits:
        nop = nc.sync.nop(nofuse=True)
        nop.ins.sync_info = bass_rust.SyncInfo(on_wait=[w], on_update=[])
    nc.sync.drain(fusable=False)

    nc.all_engine_barrier()
    assert self.sems is not None
    popped = nc._tile_sem_poison_stack.pop()
    assert popped is self._sem_poison
    nc.clear_and_free_semaphores(list(self.sems.allocated().values()))
    nc.all_engine_barrier()


bass.Bass.multi_engine_barrier = _patched_multi_engine_barrier
tile.TileContext._drain_and_barrier = _patched_drain_and_barrier


def _legalize_waits(nc, max_waits=1):
    """This walrus build rejects instructions carrying more than one sem
    wait (and any wait on a Drain). Split excess waits onto same-engine
    InstNoOp instructions inserted immediately before the instruction."""
    f = nc.m.functions[0]
    plans = []  # (block, index, inst, excess_waits, kept_waits)
    for bb in f.blocks:
        for i, inst in enumerate(bb.instructions):
            si = inst.sync_info
            if si is None:
                continue
            waits = list(si.on_wait)
            limit = 0 if type(inst).__name__ == "InstDrain" else max_waits
            if len(waits) > limit:
                plans.append((bb, inst, waits[limit:], waits[:limit],
                              list(si.on_update)))
    if not plans:
        return
    # create one nop per excess wait on the matching engine (appended to the
    # current block by the builder, then relocated)
    from collections import defaultdict
    need = defaultdict(int)
    for _, inst, excess, _, _ in plans:
        need[inst.engine] += len(excess)
    nops_by_eng = {}
    for eng_t, n in need.items():
        nops_by_eng[eng_t] = [nc.engines[eng_t].nop(nofuse=True).ins
                              for _ in range(n)]
    nopset = set()
    for lst in nops_by_eng.values():
        nopset.update(id(x) for x in lst)
    for bb in f.blocks:
        if any(id(x) in nopset for x in bb.instructions):
            bb.instructions = [x for x in bb.instructions if id(x) not in nopset]
    used = defaultdict(int)
    for bb, inst, excess, kept, updates in plans:
        inst.sync_info = bass_rust.SyncInfo(on_wait=kept, on_update=updates)
        il = bb.instructions
        pos = next(j for j, x in enumerate(il) if x is inst)
        inserts = []
        for w in excess:
            nop = nops_by_eng[inst.engine][used[inst.engine]]
            used[inst.engine] += 1
            nop.sync_info = bass_rust.SyncInfo(on_wait=[w], on_update=[])
            inserts.append(nop)
        bb.instructions = il[:pos] + inserts + il[pos:]

# ---------------------------------------------------------------------------
# constants
# ---------------------------------------------------------------------------
EPS = 1e-5
TILE = 512
BIN = 8
W = 16            # max voxel-chunks per tile
CHUNK = 64        # max points per voxel-chunk (8 bins of 8)
G = 2 * W * BIN   # gathered columns per tile
N_CORES = 8
GRID = 32         # voxel grid edge (overridable for small-scale tests)
P4_VARIANT = "base"  # engine-assignment experiments: base|actb|dver1
F32 = mybir.dt.float32
BF16 = mybir.dt.bfloat16
U16 = mybir.dt.uint16


# ---------------------------------------------------------------------------
# host-side prep
# ---------------------------------------------------------------------------
def _prepare(feats, coords, grid=None):
    if grid is None:
        grid = GRID
    """Counting sort by voxel key; occupied voxels split into <=CHUNK-point
    chunks, each chunk padded (duplicate last point) to a multiple of BIN
    slots; chunks greedily packed into TILE-slot tiles (max W chunks/tile).
    Chunks of the same voxel are max-combined on the host afterwards."""
    V = grid ** 3
    key = (coords[:, 0].astype(np.int32) * grid * grid
           + coords[:, 1].astype(np.int32) * grid
           + coords[:, 2].astype(np.int32))
    counts = np.bincount(key, minlength=V)
    order = np.argsort(key, kind="stable").astype(np.int32)
    starts = np.concatenate([[0], np.cumsum(counts)]).astype(np.int64)

    # voxel-chunks: (voxel, start_in_order, npoints)
    chunks = []
    occ = np.nonzero(counts)[0]
    for v in occ:
        c = int(counts[v]); s = int(starts[v])
        while c > 0:
            take = min(c, CHUNK)
            chunks.append((int(v), s, take))
            s += take
            c -= take

    # greedy pack into tiles
    tiles = []  # list of list-of-chunk-indices
    cur, used = [], 0
    for ci, (_, _, c) in enumerate(chunks):
        p = (c + BIN - 1) // BIN * BIN
        if used + p > TILE or len(cur) >= W:
            tiles.append(cur)
            cur, used = [], 0
        cur.append(ci)
        used += p
    if cur:
        tiles.append(cur)
    nt = len(tiles)
    t_pc = (nt + N_CORES - 1) // N_CORES
    nt_pad = t_pc * N_CORES

    idx = np.empty((nt_pad, TILE), np.int32)
    valid = np.zeros((nt_pad, TILE), np.float32)
    gidx_flat = np.zeros((nt_pad, G), np.uint16)
    kcount = np.zeros(nt_pad, np.int32)
    vox_of = np.zeros((nt_pad, W), np.int32)

    for t, cidx in enumerate(tiles):
        pos = 0
        kcount[t] = len(cidx)
        for w, ci in enumerate(cidx):
            v, s, c = chunks[ci]
            p = (c + BIN - 1) // BIN * BIN
            idx[t, pos:pos + c] = order[s:s + c]
            idx[t, pos + c:pos + p] = order[s + c - 1]
            valid[t, pos:pos + c] = 1.0
            fb = pos // BIN
            lb = (pos + p) // BIN - 1
            vox_of[t, w] = v
            for j in range(BIN):
                b = min(fb + j, lb)
                gidx_flat[t, w * BIN + j] = b
                gidx_flat[t, (W + w) * BIN + j] = TILE // BIN + b
            pos += p
        if pos < TILE:
            idx[t, pos:] = idx[t, pos - 1]
    for t in range(nt, nt_pad):
        idx[t] = idx[nt - 1]

    # wrap gather indices into the [128, G/16] layout indirect_copy expects
    s16 = G // 16
    gw = np.zeros((nt_pad, 16, s16), np.uint16)
    i_arr = np.arange(G)
    gw[:, i_arr % 16, i_arr // 16] = gidx_flat
    gidx = np.ascontiguousarray(np.tile(gw, (1, 8, 1)))

    validc = np.ascontiguousarray(valid.reshape(nt_pad, 4, 128).transpose(0, 2, 1))
    return dict(V=V, key=key, counts=counts, order=order, occ=occ, nt=nt,
                nt_pad=nt_pad, t_pc=t_pc, idx=idx, valid=valid, validc=validc,
                gidx=gidx, kcount=kcount, vox_of=vox_of)


def _feats_slabs(feats, prep):
    t_pc = prep["t_pc"]
    stream = feats[prep["idx"].reshape(-1)]  # [nt_pad*TILE, 9]
    slabs = []
    for c in range(N_CORES):
        lo, hi = c * t_pc * TILE, (c + 1) * t_pc * TILE
        s = np.empty((10, hi - lo), np.float32)
        s[0:9] = stream[lo:hi].T
        s[9] = 1.0  # ones row: folds additive terms into the L1 matmul
        slabs.append(np.ascontiguousarray(s))
    return slabs


# ---------------------------------------------------------------------------
# device programs
# ---------------------------------------------------------------------------
GSIZE = 32  # tiles per moment-accumulation group (bounds fp32 PSUM sums)


def _build_p2(t_pc):
    """x2' = relu(f @ A1aug) ; masked moments [64, 65] accumulated in PSUM,
    flushed every GSIZE tiles (host sums groups in fp64)."""
    nc = bass.Bass()
    L = t_pc * TILE
    n_grp = (t_pc + GSIZE - 1) // GSIZE
    featsT = nc.dram_tensor("featsT", [10, L], F32, kind="ExternalInput")
    validc = nc.dram_tensor("validc", [t_pc, 128, 4], F32, kind="ExternalInput")
    a1aug = nc.dram_tensor("a1aug", [10, 64], F32, kind="ExternalInput")
    mom = nc.dram_tensor("mom", [n_grp, 64, 65], F32, kind="ExternalOutput")

    with tile.TileContext(nc) as tc:
        with (
            tc.tile_pool(name="const", bufs=1) as cpool,
            tc.tile_pool(name="io", bufs=3) as io,
            tc.tile_pool(name="ps", bufs=2, space="PSUM") as ps,
            tc.tile_pool(name="accps", bufs=2, space="PSUM") as accps,
        ):
            ident = cpool.tile([128, 128], F32)
            make_identity(nc, ident[:])
            t_a1 = cpool.tile([10, 64], F32)
            nc.sync.dma_start(out=t_a1[:], in_=a1aug[:])

            for g in range(n_grp):
                t0, t1 = g * GSIZE, min((g + 1) * GSIZE, t_pc)
                p_mom = accps.tile([64, 65], F32, tag="mom")
                n_mm, last_mm = 0, 4 * (t1 - t0)
                for t in range(t0, t1):
                    xin = io.tile([10, TILE], F32, tag="xin")
                    nc.sync.dma_start(out=xin[:],
                                      in_=featsT[:, t * TILE:(t + 1) * TILE])
                    vc = io.tile([128, 4], F32, tag="vc")
                    nc.sync.dma_start(out=vc[:], in_=validc[t])
                    p_h1 = ps.tile([64, TILE], F32, tag="h1")
                    nc.tensor.matmul(out=p_h1[:], lhsT=t_a1[:], rhs=xin[:],
                                     start=True, stop=True)
                    x2 = io.tile([64, TILE], F32, tag="x2")
                    nc.scalar.activation(out=x2[:], in_=p_h1[:],
                                         func=mybir.ActivationFunctionType.Relu)
                    for c4 in range(4):
                        p_tr = ps.tile([128, 64], F32, tag="tr")
                        nc.tensor.transpose(out=p_tr[:],
                                            in_=x2[:, c4 * 128:(c4 + 1) * 128],
                                            identity=ident[:64, :64])
                        rhs = io.tile([128, 65], F32, tag="rhs")
                        # masked copy PSUM->SBUF (scale = valid column)
                        nc.vector.scalar_tensor_tensor(
                            out=rhs[:, 0:64], in0=p_tr[:], scalar=0.0,
                            in1=vc[:, c4:c4 + 1].to_broadcast([128, 64]),
                            op0=mybir.AluOpType.add, op1=mybir.AluOpType.mult)
                        nc.vector.memset(rhs[:, 64:65], 1.0)
                        nc.tensor.matmul(out=p_mom[:], lhsT=rhs[:, 0:64],
                                         rhs=rhs[:], start=(n_mm == 0),
                                         stop=(n_mm == last_mm - 1),
                                         skip_group_check=True)
                        n_mm += 1
                s_mom = io.tile([64, 65], F32, tag="smom")
                nc.vector.tensor_copy(out=s_mom[:], in_=p_mom[:])
                nc.sync.dma_start(out=mom[g], in_=s_mom[:])
    _legalize_waits(nc)
    return nc


def _build_p3(t_pc):
    """x3 = relu(x2 @ W2p + d2) (x2 = relu(f@A1aug)); masked moments
    [128, 129] flushed every GSIZE tiles."""
    nc = bass.Bass()
    L = t_pc * TILE
    n_grp = (t_pc + GSIZE - 1) // GSIZE
    featsT = nc.dram_tensor("featsT", [10, L], F32, kind="ExternalInput")
    validc = nc.dram_tensor("validc", [t_pc, 128, 4], F32, kind="ExternalInput")
    a1aug = nc.dram_tensor("a1aug", [10, 64], F32, kind="ExternalInput")
    w2 = nc.dram_tensor("w2", [64, 128], F32, kind="ExternalInput")
    d2 = nc.dram_tensor("d2", [128, 1], F32, kind="ExternalInput")
    mom = nc.dram_tensor("mom", [n_grp, 128, 129], F32, kind="ExternalOutput")

    with tile.TileContext(nc) as tc:
        with (
            tc.tile_pool(name="const", bufs=1) as cpool,
            tc.tile_pool(name="io", bufs=3) as io,
            tc.tile_pool(name="ps", bufs=2, space="PSUM") as ps,
            tc.tile_pool(name="accps", bufs=2, space="PSUM") as accps,
        ):
            ident = cpool.tile([128, 128], F32)
            make_identity(nc, ident[:])
            t_a1 = cpool.tile([10, 64], F32)
            nc.sync.dma_start(out=t_a1[:], in_=a1aug[:])
            t_w2 = cpool.tile([64, 128], F32)
            nc.sync.dma_start(out=t_w2[:], in_=w2[:])
            t_d2 = cpool.tile([128, 1], F32)
            nc.sync.dma_start(out=t_d2[:], in_=d2[:])

            for g in range(n_grp):
                t0, t1 = g * GSIZE, min((g + 1) * GSIZE, t_pc)
                p_mom = accps.tile([128, 129], F32, tag="mom")
                n_mm, last_mm = 0, 4 * (t1 - t0)
                for t in range(t0, t1):
                    xin = io.tile([10, TILE], F32, tag="xin")
                    nc.sync.dma_start(out=xin[:],
                                      in_=featsT[:, t * TILE:(t + 1) * TILE])
                    vc = io.tile([128, 4], F32, tag="vc")
                    nc.sync.dma_start(out=vc[:], in_=validc[t])
                    p_h1 = ps.tile([64, TILE], F32, tag="h1")
                    nc.tensor.matmul(out=p_h1[:], lhsT=t_a1[:], rhs=xin[:],
                                     start=True, stop=True)
                    x2 = io.tile([64, TILE], F32, tag="x2")
                    nc.scalar.activation(out=x2[:], in_=p_h1[:],
                                         func=mybir.ActivationFunctionType.Relu)
                    p_h2 = ps.tile([128, TILE], F32, tag="h2")
                    nc.tensor.matmul(out=p_h2[:], lhsT=t_w2[:], rhs=x2[:],
                                     start=True, stop=True)
                    x3 = io.tile([128, TILE], F32, tag="x3")
                    nc.scalar.activation(out=x3[:], in_=p_h2[:],
                                         func=mybir.ActivationFunctionType.Relu,
                                         bias=t_d2[:, 0:1])
                    for c4 in range(4):
                        p_tr = ps.tile([128, 128], F32, tag="tr")
                        nc.tensor.transpose(out=p_tr[:],
                                            in_=x3[:, c4 * 128:(c4 + 1) * 128],
                                            identity=ident[:])
                        rhs = io.tile([128, 129], F32, tag="rhs")
                        nc.vector.scalar_tensor_tensor(
                            out=rhs[:, 0:128], in0=p_tr[:], scalar=0.0,
                            in1=vc[:, c4:c4 + 1].to_broadcast([128, 128]),
                            op0=mybir.AluOpType.add, op1=mybir.AluOpType.mult)
                        nc.vector.memset(rhs[:, 128:129], 1.0)
                        nc.tensor.matmul(out=p_mom[:], lhsT=rhs[:, 0:128],
                                         rhs=rhs[:], start=(n_mm == 0),
                                         stop=(n_mm == last_mm - 1),
                                         skip_group_check=True)
                        n_mm += 1
                s_mom = io.tile([128, 129], F32, tag="smom")
                nc.vector.tensor_copy(out=s_mom[:], in_=p_mom[:])
                nc.sync.dma_start(out=mom[g], in_=s_mom[:])
    _legalize_waits(nc)
    return nc


def _build_p4(t_pc):
    """Full MLP + segmented max via bin-reduce + clamped gather."""
    nc = bass.Bass()
    L = t_pc * TILE
    featsT = nc.dram_tensor("featsT", [10, L], F32, kind="ExternalInput")
    gidx = nc.dram_tensor("gidx", [t_pc, 128, G // 16], U16, kind="ExternalInput")
    a1aug = nc.dram_tensor("a1aug", [10, 64], F32, kind="ExternalInput")
    w2 = nc.dram_tensor("w2", [64, 128], F32, kind="ExternalInput")
    d2 = nc.dram_tensor("d2", [128, 1], F32, kind="ExternalInput")
    w3 = nc.dram_tensor("w3", [128, 256], F32, kind="ExternalInput")
    d3 = nc.dram_tensor("d3", [256, 1], F32, kind="ExternalInput")
    w4 = nc.dram_tensor("w4", [256, 256], F32, kind="ExternalInput")
    out = nc.dram_tensor("out", [128, t_pc * 2 * W], F32, kind="ExternalOutput")

    RELU = mybir.ActivationFunctionType.Relu
    with tile.TileContext(nc) as tc:
        with (
            tc.tile_pool(name="const", bufs=1) as cpool,
            tc.tile_pool(name="io", bufs=4) as io,
            tc.tile_pool(name="psA", bufs=2, space="PSUM") as psA,
            tc.tile_pool(name="ps", bufs=1, space="PSUM") as ps,
        ):
            t_a1 = cpool.tile([10, 64], F32)
            nc.sync.dma_start(out=t_a1[:], in_=a1aug[:])
            t_w2 = cpool.tile([64, 128], F32)
            nc.sync.dma_start(out=t_w2[:], in_=w2[:])
            t_d2 = cpool.tile([128, 1], F32)
            nc.sync.dma_start(out=t_d2[:], in_=d2[:])
            t_w3 = cpool.tile([128, 256], F32)
            nc.sync.dma_start(out=t_w3[:], in_=w3[:])
            # d3 [256,1] stored as [128, 2]: col0 = d3[:128], col1 = d3[128:]
            t_d3 = cpool.tile([128, 2], F32)
            nc.sync.dma_start(out=t_d3[:, 0:1], in_=d3[0:128, :])
            nc.sync.dma_start(out=t_d3[:, 1:2], in_=d3[128:256, :])
            t_negd3b = cpool.tile([128, 1], F32)
            nc.vector.tensor_scalar_mul(out=t_negd3b[:], in0=t_d3[:, 1:2], scalar1=-1.0)
            # w4 [256, 256] stored as [128, 512]: cols 0:256 = w4[0:128],
            # cols 256:512 = w4[128:256]
            t_w4 = cpool.tile([128, 512], F32)
            # w4 [256, 256] -> [128, 512]: rows 0:128 cols 0:256 = w4[0:128],
            # cols 256:512 = w4[128:256]
            nc.sync.dma_start(out=t_w4[:, 0:256], in_=w4[0:128, :])
            nc.sync.dma_start(out=t_w4[:, 256:512], in_=w4[128:256, :])

            for t in range(t_pc):
                xin = io.tile([10, TILE], F32, tag="xin")
                nc.sync.dma_start(out=xin[:], in_=featsT[:, t * TILE:(t + 1) * TILE])
                t_gi = io.tile([128, G // 16], U16, tag="gi")
                nc.sync.dma_start(out=t_gi[:], in_=gidx[t])

                p_h1 = ps.tile([64, TILE], F32, tag="h1")
                nc.tensor.matmul(out=p_h1[:], lhsT=t_a1[:], rhs=xin[:],
                                 start=True, stop=True)
                x2 = io.tile([64, TILE], F32, tag="x2")
                nc.scalar.activation(out=x2[:], in_=p_h1[:], func=RELU)

                p_h2 = ps.tile([128, TILE], F32, tag="h2")
                nc.tensor.matmul(out=p_h2[:], lhsT=t_w2[:], rhs=x2[:],
                                 start=True, stop=True)
                x3 = io.tile([128, TILE], F32, tag="x3")
                nc.scalar.activation(out=x3[:], in_=p_h2[:], func=RELU,
                                     bias=t_d2[:, 0:1])

                p_h3a = ps.tile([128, TILE], F32, tag="h3a")
                nc.tensor.matmul(out=p_h3a[:], lhsT=t_w3[:, 0:128], rhs=x3[:],
                                 start=True, stop=True)
                p_h3b = ps.tile([128, TILE], F32, tag="h3b")
                nc.tensor.matmul(out=p_h3b[:], lhsT=t_w3[:, 128:256], rhs=x3[:],
                                 start=True, stop=True)
                x4a = io.tile([128, TILE], F32, tag="x4a")
                nc.scalar.activation(out=x4a[:], in_=p_h3a[:], func=RELU,
                                     bias=t_d3[:, 0:1])
                x4b = io.tile([128, TILE], F32, tag="x4b")
                if P4_VARIANT == "actb":
                    # all relus on ACT; no shift (b4 fold excludes d3b term)
                    nc.scalar.activation(out=x4b[:], in_=p_h3b[:], func=RELU,
                                         bias=t_d3[:, 1:2])
                else:
                    # x4b' = max(h3b, -d3b) (shift; fold d3b@W4b into b4)
                    nc.vector.scalar_tensor_tensor(
                        out=x4b[:], in0=p_h3b[:], scalar=0.0,
                        in1=t_negd3b[:, 0:1].to_broadcast([128, TILE]),
                        op0=mybir.AluOpType.add, op1=mybir.AluOpType.max)

                p_h4 = psA.tile([128, 2 * TILE], F32, tag="h4")
                nc.tensor.matmul(out=p_h4[:, 0:TILE], lhsT=t_w4[:, 0:128],
                                 rhs=x4a[:], start=True, stop=False,
                                 skip_group_check=True)
                nc.tensor.matmul(out=p_h4[:, 0:TILE], lhsT=t_w4[:, 256:384],
                                 rhs=x4b[:], start=False, stop=True,
                                 skip_group_check=True)
                nc.tensor.matmul(out=p_h4[:, TILE:2 * TILE], lhsT=t_w4[:, 128:256],
                                 rhs=x4a[:], start=True, stop=False,
                                 skip_group_check=True)
                nc.tensor.matmul(out=p_h4[:, TILE:2 * TILE], lhsT=t_w4[:, 384:512],
                                 rhs=x4b[:], start=False, stop=True,
                                 skip_group_check=True)

                binmax = io.tile([128, 2 * TILE // BIN], F32, tag="binmax")
                nc.vector.tensor_reduce(
                    out=binmax[:],
                    in_=p_h4[:].rearrange("p (b e) -> p b e", e=BIN),
                    axis=mybir.AxisListType.X, op=mybir.AluOpType.max)

                gat = io.tile([128, G], F32, tag="gat")
                nc.gpsimd.indirect_copy(out=gat[:], data=binmax[:], idxs=t_gi[:],
                                        i_know_ap_gather_is_preferred=True)
                vox = io.tile([128, 2 * W], F32, tag="vox")
                nc.vector.tensor_reduce(
                    out=vox[:], in_=gat[:].rearrange("p (w e) -> p w e", e=BIN),
                    axis=mybir.AxisListType.X, op=mybir.AluOpType.max)
                nc.sync.dma_start(out=out[:, t * 2 * W:(t + 1) * 2 * W], in_=vox[:])
    _legalize_waits(nc)
    return nc


# ---------------------------------------------------------------------------
# host folding math (fp64)
# ---------------------------------------------------------------------------
def _fold(inputs, prep):
    """P1 on host: BN0 fold + analytic BN1 stats -> A1aug."""
    feats = np.asarray(inputs["feats"], np.float32)
    N = feats.shape[0]
    g0 = np.asarray(inputs["g0"], np.float64); be0 = np.asarray(inputs["be0"], np.float64)
    W1 = np.asarray(inputs["W1"], np.float64); b1 = np.asarray(inputs["b1"], np.float64)
    g1 = np.asarray(inputs["g1"], np.float64); be1 = np.asarray(inputs["be1"], np.float64)

    f64 = feats.astype(np.float64)
    m0 = f64.mean(0)
    M2f = f64.T @ f64 / N
    v0 = np.diag(M2f) - m0 * m0
    a0 = g0 / np.sqrt(v0 + EPS)
    A1 = W1 * a0[:, None]
    d1 = (be0 - m0 * a0) @ W1 + b1
    m1 = m0 @ A1 + d1
    Cov = M2f - np.outer(m0, m0)
    v1 = np.einsum("ij,ik,kj->j", A1, Cov, A1)
    a1 = g1 / np.sqrt(v1 + EPS)
    A1p = A1 * a1[None, :]
    d1p = (d1 - m1) * a1 + be1
    a1aug = np.empty((10, 64), np.float32)
    a1aug[0:9] = A1p.astype(np.float32)
    a1aug[9] = d1p.astype(np.float32)
    return dict(N=N, a1aug=a1aug)


def _fold2(mom2, N, inputs):
    """host fold after P2: moments of x2 -> W2aug(d2) for BN2."""
    W2 = np.asarray(inputs["W2"], np.float64); b2 = np.asarray(inputs["b2"], np.float64)
    g2 = np.asarray(inputs["g2"], np.float64); be2 = np.asarray(inputs["be2"], np.float64)
    S2 = mom2[:, :64].astype(np.float64)
    s2 = mom2[:, 64].astype(np.float64)
    m2 = s2 / N
    Cov2 = S2 / N - np.outer(m2, m2)
    m2h = m2 @ W2 + b2
    v2h = np.einsum("ij,ik,kj->j", W2, Cov2, W2)
    a2 = g2 / np.sqrt(v2h + EPS)
    W2p = (W2 * a2[None, :]).astype(np.float32)
    d2p = ((b2 - m2h) * a2 + be2).astype(np.float32)
    return W2p, d2p


def _fold3(mom3, N, d2p, inputs):
    """host fold after P3: moments of x3' (= x3 - d2p... see note) -> BN3.

    P3 computes x3 = relu(h2 + d2p) directly (bias via ACT), masked; so
    mom3 are moments of the true x3. Derive BN3 fold.
    """
    W3 = np.asarray(inputs["W3"], np.float64); b3 = np.asarray(inputs["b3"], np.float64)
    g3 = np.asarray(inputs["g3"], np.float64); be3 = np.asarray(inputs["be3"], np.float64)
    S3 = mom3[:, :128].astype(np.float64)
    s3 = mom3[:, 128].astype(np.float64)
    m3 = s3 / N
    Cov3 = S3 / N - np.outer(m3, m3)
    m3h = m3 @ W3 + b3
    v3h = np.einsum("ij,ik,kj->j", W3, Cov3, W3)
    a3 = g3 / np.sqrt(v3h + EPS)
    W3p = (W3 * a3[None, :]).astype(np.float32)
    d3p = ((b3 - m3h) * a3 + be3).astype(np.float32)
    return W3p, d3p


# ---------------------------------------------------------------------------
# main entry
# ---------------------------------------------------------------------------
import os as _os
import time as _time

LAST_STATS = {}


def _run_spmd(tag, nc, in_maps, core_ids):
    # NTFF profiling is unavailable under this axon client (no
    # antenv.axon_hooks), so when BASS_KERNEL_TRACE is set we also time a
    # second, jit-cached execution as the device-time proxy (includes axon
    # RPC + host<->device transfer, so it upper-bounds the NEFF time).
    t0 = _time.time()
    res = run_bass_kernel_spmd(nc, in_maps, core_ids=core_ids)
    wall1 = _time.time() - t0
    exec_wall2 = None
    if int(_os.environ.get("BASS_KERNEL_TRACE", "0")):
        t0 = _time.time()
        run_bass_kernel_spmd(nc, in_maps, core_ids=core_ids)
        exec_wall2 = _time.time() - t0
    LAST_STATS[tag] = dict(wall_s=wall1,
                           exec_ns=getattr(res, "exec_time_ns", None),
                           exec_wall2_s=exec_wall2)
    return res


def kernel(**inputs):
    feats = np.ascontiguousarray(np.asarray(inputs["feats"], np.float32))
    coords = np.ascontiguousarray(np.asarray(inputs["coords"], np.int32))
    num_voxels = int(np.asarray(inputs["num_voxels"]))
    prep = _prepare(feats, coords)
    V = prep["V"]
    assert V == num_voxels
    t_pc = prep["t_pc"]
    fold1 = _fold(inputs, prep)
    N = fold1["N"]
    a1aug = fold1["a1aug"]
    slabs = _feats_slabs(feats, prep)
    validc = prep["validc"].reshape(N_CORES, t_pc, 128, 4)
    gidx = prep["gidx"].reshape(N_CORES, t_pc, 128, G // 16)

    core_ids = list(range(N_CORES))

    # ---- P2
    nc2 = _build_p2(t_pc)
    in2 = [{"featsT": slabs[c], "validc": np.ascontiguousarray(validc[c]),
            "a1aug": a1aug} for c in core_ids]
    res2 = _run_spmd("p2", nc2, in2, core_ids)
    mom2 = np.sum([r["mom"].astype(np.float64).sum(0) for r in res2.results], axis=0)
    W2p, d2p = _fold2(mom2, N, inputs)

    # ---- P3
    nc3 = _build_p3(t_pc)
    d2col = np.ascontiguousarray(d2p.reshape(128, 1))
    in3 = [{"featsT": slabs[c], "validc": np.ascontiguousarray(validc[c]),
            "a1aug": a1aug, "w2": W2p, "d2": d2col} for c in core_ids]
    res3 = _run_spmd("p3", nc3, in3, core_ids)
    mom3 = np.sum([r["mom"].astype(np.float64).sum(0) for r in res3.results], axis=0)
    W3p, d3p = _fold3(mom3, N, d2p, inputs)

    # ---- P4
    W4 = np.asarray(inputs["W4"], np.float32)
    b4 = np.asarray(inputs["b4"], np.float64)
    # x4b' = max(h3b, -d3b) = x4b - d3b  =>  fold d3b @ W4[128:] into bias
    b4_eff = (b4 + d3p[128:].astype(np.float64) @ W4[128:256].astype(np.float64))
    nc4 = _build_p4(t_pc)
    d3col = np.ascontiguousarray(d3p.reshape(256, 1))
    in4 = [{"featsT": slabs[c], "gidx": np.ascontiguousarray(gidx[c]),
            "a1aug": a1aug, "w2": W2p, "d2": d2col,
            "w3": np.ascontiguousarray(W3p), "d3": d3col,
            "w4": np.ascontiguousarray(W4)} for c in core_ids]
    res4 = _run_spmd("p4", nc4, in4, core_ids)

    # ---- assemble: rows follow jnp.unique(sorted occupied keys) order,
    # chunk slots of the same voxel max-combined; padding rows are empty
    # segments (-inf) with fill_value=0 coords.
    full = np.concatenate([r["out"] for r in res4.results], axis=1)  # [128, nt_pad*2W]
    nt = prep["nt"]
    kc = prep["kcount"][:nt]
    tt, ww = np.nonzero(np.arange(W)[None, :] < kc[:, None])
    vids = prep["vox_of"][tt, ww]
    colA = tt * 2 * W + ww
    colB = colA + W
    occ = prep["occ"]
    # vids follows occ order (chunks emitted in sorted-voxel order), so
    # same-voxel chunk slots are contiguous: max-combine via reduceat.
    assert np.all(np.diff(vids) >= 0)
    grp = np.flatnonzero(np.concatenate([[True], vids[1:] != vids[:-1]]))
    assert len(grp) == len(occ)
    pooled = np.full((V, 256), -np.inf, np.float32)
    pooled[:len(occ), 0:128] = np.maximum.reduceat(full[:, colA].T, grp, axis=0)
    pooled[:len(occ), 128:256] = np.maximum.reduceat(full[:, colB].T, grp, axis=0)
    pooled[:len(occ)] += b4_eff[None, :].astype(np.float32)

    unq_coords = np.zeros((V, 3), np.int64)
    unq_coords[:len(occ), 0] = occ // (GRID * GRID)
    unq_coords[:len(occ), 1] = (occ // GRID) % GRID
    unq_coords[:len(occ), 2] = occ % GRID
    return pooled.astype(np.float32), unq_coords


# revision 32
# speedup vs baseline: 1.1267x; 1.1267x over previous
"""Trainium2 Bass kernel for nn_CylindricalPointEncoder (segment_reduce).

Pipeline (8 NeuronCores, SPMD):
  host: voxel-key counting sort; voxels split into <=64-point chunks,
        padded (duplicate points) to multiples of 8 slots; greedy-packed
        into 512-point tiles (max 16 chunks/tile); per-core slabs of
        transposed features.
  P1 (host, tiny): moments of raw feats -> BN0 fold + analytic BN1 stats.
  P2 (device): x2 = relu(f @ A1 + d1) ; masked moments [64, 65] -> host
        folds BN2.
  P3 (device): x3 moments [128, 129] -> host folds BN3.
  P4 (device): full MLP -> h4 [256 x 512] per tile -> 8-slot bin max
        (DVE reduce) -> one wide indirect_copy gather of clamped bin
        indices -> per-voxel max -> compact output columns.
  host: reassemble [V, 256] (+ folded bias), emit unq_coords.

BN folding: y = BN(x) @ W + b == x @ (a*W) + ((be - m*a) @ W + b) with
a = g/sqrt(v+eps); stats of pre-activations derived analytically from
moments of the (relu'd) previous layer, so only two device stat passes
are needed. relu biases are folded into the next layer's additive term
(x' = max(h, -d); x = x' + d) where convenient.
"""

import math
import numpy as np

# ---------------------------------------------------------------------------
# compat: the walrus build in this container rejects InstDrain carrying sem
# waits/updates on PE/ACT/DVE/SP ("Too many sync wait commands"). Replace the
# tile barrier + tail drain with split nop/sem_inc/wait_ge sequences.
# ---------------------------------------------------------------------------
import bass_rust
import concourse.bass as bass
import concourse.mybir as mybir
import concourse.tile as tile
from concourse.vector_clock import ScopedClock
from concourse.bass_utils import run_bass_kernel_spmd
from concourse.masks import make_identity


def _ensure_barrier_sem(nc):
    if getattr(nc, "_compat_aeb_sem", None) is None:
        nc._compat_aeb_sem = nc.alloc_semaphore("compat_aeb_sem")
        nc._compat_aeb_count = 0
    return nc._compat_aeb_sem


def _patched_multi_engine_barrier(self, engines):
    sem = _ensure_barrier_sem(self)
    target = self._compat_aeb_count + len(engines)
    for eng_t in engines:
        eng = self.engines[eng_t]
        eng.drain(fusable=False)
        eng.sem_inc(sem, 1)
    for eng_t in engines:
        self.engines[eng_t].wait_ge(sem, target)
    self._compat_aeb_count = target


def _patched_drain_and_barrier(self, tick_clock, wait_clock):
    nc = self.nc
    probe = nc.sync.nop(nofuse=True)
    wait_clock.add_sem_waits(probe.ins, ScopedClock({None: tick_clock.global_clock}))
    si = probe.ins.sync_info
    waits = list(si.on_wait) if si is not None else []
    probe.ins.sync_info = bass_rust.SyncInfo(on_wait=[], on_update=[])
    for w in waits:
        nop = nc.sync.nop(nofuse=True)
        nop.ins.sync_info = bass_rust.SyncInfo(on_wait=[w], on_update=[])
    nc.sync.drain(fusable=False)

    nc.all_engine_barrier()
    assert self.sems is not None
    popped = nc._tile_sem_poison_stack.pop()
    assert popped is self._sem_poison
    nc.clear_and_free_semaphores(list(self.sems.allocated().values()))
    nc.all_engine_barrier()


bass.Bass.multi_engine_barrier = _patched_multi_engine_barrier
tile.TileContext._drain_and_barrier = _patched_drain_and_barrier


def _legalize_waits(nc, max_waits=1):
    """This walrus build rejects instructions carrying more than one sem
    wait (and any wait on a Drain). Split excess waits onto same-engine
    InstNoOp instructions inserted immediately before the instruction."""
    f = nc.m.functions[0]
    plans = []  # (block, index, inst, excess_waits, kept_waits)
    for bb in f.blocks:
        for i, inst in enumerate(bb.instructions):
            si = inst.sync_info
            if si is None:
                continue
            waits = list(si.on_wait)
            limit = 0 if type(inst).__name__ == "InstDrain" else max_waits
            if len(waits) > limit:
                plans.append((bb, inst, waits[limit:], waits[:limit],
                              list(si.on_update)))
    if not plans:
        return
    # create one nop per excess wait on the matching engine (appended to the
    # current block by the builder, then relocated)
    from collections import defaultdict
    need = defaultdict(int)
    for _, inst, excess, _, _ in plans:
        need[inst.engine] += len(excess)
    nops_by_eng = {}
    for eng_t, n in need.items():
        nops_by_eng[eng_t] = [nc.engines[eng_t].nop(nofuse=True).ins
                              for _ in range(n)]
    nopset = set()
    for lst in nops_by_eng.values():
        nopset.update(id(x) for x in lst)
    for bb in f.blocks:
        if any(id(x) in nopset for x in bb.instructions):
            bb.instructions = [x for x in bb.instructions if id(x) not in nopset]
    used = defaultdict(int)
    for bb, inst, excess, kept, updates in plans:
        inst.sync_info = bass_rust.SyncInfo(on_wait=kept, on_update=updates)
        il = bb.instructions
        pos = next(j for j, x in enumerate(il) if x is inst)
        inserts = []
        for w in excess:
            nop = nops_by_eng[inst.engine][used[inst.engine]]
            used[inst.engine] += 1
            nop.sync_info = bass_rust.SyncInfo(on_wait=[w], on_update=[])
            inserts.append(nop)
        bb.instructions = il[:pos] + inserts + il[pos:]

# ---------------------------------------------------------------------------
# constants
# ---------------------------------------------------------------------------
EPS = 1e-5
TILE = 512
BIN = 8
W = 16            # max voxel-chunks per tile
CHUNK = 64        # max points per voxel-chunk (8 bins of 8)
G = 2 * W * BIN   # gathered columns per tile
N_CORES = 8
GRID = 32         # voxel grid edge (overridable for small-scale tests)
P4_VARIANT = "base"  # engine-assignment experiments: base|actb|dver1
F32 = mybir.dt.float32
BF16 = mybir.dt.bfloat16
U16 = mybir.dt.uint16


# ---------------------------------------------------------------------------
# host-side prep
# ---------------------------------------------------------------------------
def _prepare(feats, coords, grid=None):
    if grid is None:
        grid = GRID
    """Counting sort by voxel key; occupied voxels split into <=CHUNK-point
    chunks, each chunk padded (duplicate last point) to a multiple of BIN
    slots; chunks greedily packed into TILE-slot tiles (max W chunks/tile).
    Chunks of the same voxel are max-combined on the host afterwards."""
    V = grid ** 3
    key = (coords[:, 0].astype(np.int32) * grid * grid
           + coords[:, 1].astype(np.int32) * grid
           + coords[:, 2].astype(np.int32))
    counts = np.bincount(key, minlength=V)
    order = np.argsort(key, kind="stable").astype(np.int32)
    starts = np.concatenate([[0], np.cumsum(counts)]).astype(np.int64)

    # voxel-chunks: (voxel, start_in_order, npoints)
    chunks = []
    occ = np.nonzero(counts)[0]
    for v in occ:
        c = int(counts[v]); s = int(starts[v])
        while c > 0:
            take = min(c, CHUNK)
            chunks.append((int(v), s, take))
            s += take
            c -= take

    # greedy pack into tiles
    tiles = []  # list of list-of-chunk-indices
    cur, used = [], 0
    for ci, (_, _, c) in enumerate(chunks):
        p = (c + BIN - 1) // BIN * BIN
        if used + p > TILE or len(cur) >= W:
            tiles.append(cur)
            cur, used = [], 0
        cur.append(ci)
        used += p
    if cur:
        tiles.append(cur)
    nt = len(tiles)
    t_pc = (nt + N_CORES - 1) // N_CORES
    nt_pad = t_pc * N_CORES

    idx = np.empty((nt_pad, TILE), np.int32)
    valid = np.zeros((nt_pad, TILE), np.float32)
    gidx_flat = np.zeros((nt_pad, G), np.uint16)
    kcount = np.zeros(nt_pad, np.int32)
    vox_of = np.zeros((nt_pad, W), np.int32)

    for t, cidx in enumerate(tiles):
        pos = 0
        kcount[t] = len(cidx)
        for w, ci in enumerate(cidx):
            v, s, c = chunks[ci]
            p = (c + BIN - 1) // BIN * BIN
            idx[t, pos:pos + c] = order[s:s + c]
            idx[t, pos + c:pos + p] = order[s + c - 1]
            valid[t, pos:pos + c] = 1.0
            fb = pos // BIN
            lb = (pos + p) // BIN - 1
            vox_of[t, w] = v
            for j in range(BIN):
                b = min(fb + j, lb)
                gidx_flat[t, w * BIN + j] = b
                gidx_flat[t, (W + w) * BIN + j] = TILE // BIN + b
            pos += p
        if pos < TILE:
            idx[t, pos:] = idx[t, pos - 1]
    for t in range(nt, nt_pad):
        idx[t] = idx[nt - 1]

    # wrap gather indices into the [128, G/16] layout indirect_copy expects
    s16 = G // 16
    gw = np.zeros((nt_pad, 16, s16), np.uint16)
    i_arr = np.arange(G)
    gw[:, i_arr % 16, i_arr // 16] = gidx_flat
    gidx = np.ascontiguousarray(np.tile(gw, (1, 8, 1)))

    validc = np.ascontiguousarray(valid.reshape(nt_pad, 4, 128).transpose(0, 2, 1))
    return dict(V=V, key=key, counts=counts, order=order, occ=occ, nt=nt,
                nt_pad=nt_pad, t_pc=t_pc, idx=idx, valid=valid, validc=validc,
                gidx=gidx, kcount=kcount, vox_of=vox_of)


def _feats_slabs(feats, prep):
    t_pc = prep["t_pc"]
    stream = feats[prep["idx"].reshape(-1)]  # [nt_pad*TILE, 9]
    slabs = []
    for c in range(N_CORES):
        lo, hi = c * t_pc * TILE, (c + 1) * t_pc * TILE
        s = np.empty((10, hi - lo), np.float32)
        s[0:9] = stream[lo:hi].T
        s[9] = 1.0  # ones row: folds additive terms into the L1 matmul
        slabs.append(np.ascontiguousarray(s))
    return slabs


# ---------------------------------------------------------------------------
# device programs
# ---------------------------------------------------------------------------
GSIZE = 32  # tiles per moment-accumulation group (bounds fp32 PSUM sums)


def _build_p2(t_pc):
    """x2' = relu(f @ A1aug) ; masked moments [64, 65] accumulated in PSUM,
    flushed every GSIZE tiles (host sums groups in fp64)."""
    nc = bass.Bass()
    L = t_pc * TILE
    n_grp = (t_pc + GSIZE - 1) // GSIZE
    featsT = nc.dram_tensor("featsT", [10, L], F32, kind="ExternalInput")
    validc = nc.dram_tensor("validc", [t_pc, 128, 4], F32, kind="ExternalInput")
    a1aug = nc.dram_tensor("a1aug", [10, 64], F32, kind="ExternalInput")
    mom = nc.dram_tensor("mom", [n_grp, 64, 65], F32, kind="ExternalOutput")

    with tile.TileContext(nc) as tc:
        with (
            tc.tile_pool(name="const", bufs=1) as cpool,
            tc.tile_pool(name="io", bufs=3) as io,
            tc.tile_pool(name="ps", bufs=2, space="PSUM") as ps,
            tc.tile_pool(name="accps", bufs=2, space="PSUM") as accps,
        ):
            ident = cpool.tile([128, 128], F32)
            make_identity(nc, ident[:])
            t_a1 = cpool.tile([10, 64], F32)
            nc.sync.dma_start(out=t_a1[:], in_=a1aug[:])

            for g in range(n_grp):
                t0, t1 = g * GSIZE, min((g + 1) * GSIZE, t_pc)
                p_mom = accps.tile([64, 65], F32, tag="mom")
                n_mm, last_mm = 0, 4 * (t1 - t0)
                for t in range(t0, t1):
                    xin = io.tile([10, TILE], F32, tag="xin")
                    nc.sync.dma_start(out=xin[:],
                                      in_=featsT[:, t * TILE:(t + 1) * TILE])
                    vc = io.tile([128, 4], F32, tag="vc")
                    nc.sync.dma_start(out=vc[:], in_=validc[t])
                    p_h1 = ps.tile([64, TILE], F32, tag="h1")
                    nc.tensor.matmul(out=p_h1[:], lhsT=t_a1[:], rhs=xin[:],
                                     start=True, stop=True)
                    x2 = io.tile([64, TILE], F32, tag="x2")
                    nc.scalar.activation(out=x2[:], in_=p_h1[:],
                                         func=mybir.ActivationFunctionType.Relu)
                    for c4 in range(4):
                        p_tr = ps.tile([128, 64], F32, tag="tr")
                        nc.tensor.transpose(out=p_tr[:],
                                            in_=x2[:, c4 * 128:(c4 + 1) * 128],
                                            identity=ident[:64, :64])
                        rhs = io.tile([128, 65], F32, tag="rhs")
                        # masked copy PSUM->SBUF (scale = valid column)
                        nc.vector.scalar_tensor_tensor(
                            out=rhs[:, 0:64], in0=p_tr[:], scalar=0.0,
                            in1=vc[:, c4:c4 + 1].to_broadcast([128, 64]),
                            op0=mybir.AluOpType.add, op1=mybir.AluOpType.mult)
                        nc.vector.memset(rhs[:, 64:65], 1.0)
                        nc.tensor.matmul(out=p_mom[:], lhsT=rhs[:, 0:64],
                                         rhs=rhs[:], start=(n_mm == 0),
                                         stop=(n_mm == last_mm - 1),
                                         skip_group_check=True)
                        n_mm += 1
                s_mom = io.tile([64, 65], F32, tag="smom")
                nc.vector.tensor_copy(out=s_mom[:], in_=p_mom[:])
                nc.sync.dma_start(out=mom[g], in_=s_mom[:])
    _legalize_waits(nc)
    return nc


def _build_p3(t_pc):
    """x3 = relu(x2 @ W2p + d2) (x2 = relu(f@A1aug)); masked moments
    [128, 129] flushed every GSIZE tiles."""
    nc = bass.Bass()
    L = t_pc * TILE
    n_grp = (t_pc + GSIZE - 1) // GSIZE
    featsT = nc.dram_tensor("featsT", [10, L], F32, kind="ExternalInput")
    validc = nc.dram_tensor("validc", [t_pc, 128, 4], F32, kind="ExternalInput")
    a1aug = nc.dram_tensor("a1aug", [10, 64], F32, kind="ExternalInput")
    w2 = nc.dram_tensor("w2", [64, 128], F32, kind="ExternalInput")
    d2 = nc.dram_tensor("d2", [128, 1], F32, kind="ExternalInput")
    mom = nc.dram_tensor("mom", [n_grp, 128, 129], F32, kind="ExternalOutput")

    with tile.TileContext(nc) as tc:
        with (
            tc.tile_pool(name="const", bufs=1) as cpool,
            tc.tile_pool(name="io", bufs=3) as io,
            tc.tile_pool(name="ps", bufs=2, space="PSUM") as ps,
            tc.tile_pool(name="accps", bufs=2, space="PSUM") as accps,
        ):
            ident = cpool.tile([128, 128], F32)
            make_identity(nc, ident[:])
            t_a1 = cpool.tile([10, 64], F32)
            nc.sync.dma_start(out=t_a1[:], in_=a1aug[:])
            t_w2 = cpool.tile([64, 128], F32)
            nc.sync.dma_start(out=t_w2[:], in_=w2[:])
            t_d2 = cpool.tile([128, 1], F32)
            nc.sync.dma_start(out=t_d2[:], in_=d2[:])

            for g in range(n_grp):
                t0, t1 = g * GSIZE, min((g + 1) * GSIZE, t_pc)
                p_mom = accps.tile([128, 129], F32, tag="mom")
                n_mm, last_mm = 0, 4 * (t1 - t0)
                for t in range(t0, t1):
                    xin = io.tile([10, TILE], F32, tag="xin")
                    nc.sync.dma_start(out=xin[:],
                                      in_=featsT[:, t * TILE:(t + 1) * TILE])
                    vc = io.tile([128, 4], F32, tag="vc")
                    nc.sync.dma_start(out=vc[:], in_=validc[t])
                    p_h1 = ps.tile([64, TILE], F32, tag="h1")
                    nc.tensor.matmul(out=p_h1[:], lhsT=t_a1[:], rhs=xin[:],
                                     start=True, stop=True)
                    x2 = io.tile([64, TILE], F32, tag="x2")
                    nc.scalar.activation(out=x2[:], in_=p_h1[:],
                                         func=mybir.ActivationFunctionType.Relu)
                    p_h2 = ps.tile([128, TILE], F32, tag="h2")
                    nc.tensor.matmul(out=p_h2[:], lhsT=t_w2[:], rhs=x2[:],
                                     start=True, stop=True)
                    x3 = io.tile([128, TILE], F32, tag="x3")
                    nc.scalar.activation(out=x3[:], in_=p_h2[:],
                                         func=mybir.ActivationFunctionType.Relu,
                                         bias=t_d2[:, 0:1])
                    for c4 in range(4):
                        p_tr = ps.tile([128, 128], F32, tag="tr")
                        nc.tensor.transpose(out=p_tr[:],
                                            in_=x3[:, c4 * 128:(c4 + 1) * 128],
                                            identity=ident[:])
                        rhs = io.tile([128, 129], F32, tag="rhs")
                        nc.vector.scalar_tensor_tensor(
                            out=rhs[:, 0:128], in0=p_tr[:], scalar=0.0,
                            in1=vc[:, c4:c4 + 1].to_broadcast([128, 128]),
                            op0=mybir.AluOpType.add, op1=mybir.AluOpType.mult)
                        nc.vector.memset(rhs[:, 128:129], 1.0)
                        nc.tensor.matmul(out=p_mom[:], lhsT=rhs[:, 0:128],
                                         rhs=rhs[:], start=(n_mm == 0),
                                         stop=(n_mm == last_mm - 1),
                                         skip_group_check=True)
                        n_mm += 1
                s_mom = io.tile([128, 129], F32, tag="smom")
                nc.vector.tensor_copy(out=s_mom[:], in_=p_mom[:])
                nc.sync.dma_start(out=mom[g], in_=s_mom[:])
    _legalize_waits(nc)
    return nc


def _build_p4(t_pc):
    """Full MLP + segmented max via bin-reduce + clamped gather."""
    nc = bass.Bass()
    L = t_pc * TILE
    featsT = nc.dram_tensor("featsT", [10, L], BF16, kind="ExternalInput")
    gidx = nc.dram_tensor("gidx", [t_pc, 128, G // 16], U16, kind="ExternalInput")
    a1aug = nc.dram_tensor("a1aug", [10, 64], BF16, kind="ExternalInput")
    w2 = nc.dram_tensor("w2", [64, 128], BF16, kind="ExternalInput")
    d2 = nc.dram_tensor("d2", [128, 1], F32, kind="ExternalInput")
    w3 = nc.dram_tensor("w3", [128, 256], BF16, kind="ExternalInput")
    d3 = nc.dram_tensor("d3", [256, 1], F32, kind="ExternalInput")
    w4 = nc.dram_tensor("w4", [256, 256], BF16, kind="ExternalInput")
    out = nc.dram_tensor("out", [128, t_pc * 2 * W], F32, kind="ExternalOutput")

    RELU = mybir.ActivationFunctionType.Relu
    with tile.TileContext(nc) as tc:
        with (
            tc.tile_pool(name="const", bufs=1) as cpool,
            tc.tile_pool(name="io", bufs=4) as io,
            tc.tile_pool(name="psA", bufs=2, space="PSUM") as psA,
            tc.tile_pool(name="ps", bufs=1, space="PSUM") as ps,
        ):
            t_a1 = cpool.tile([10, 64], BF16)
            nc.sync.dma_start(out=t_a1[:], in_=a1aug[:])
            t_w2 = cpool.tile([64, 128], BF16)
            nc.sync.dma_start(out=t_w2[:], in_=w2[:])
            t_d2 = cpool.tile([128, 1], F32)
            nc.sync.dma_start(out=t_d2[:], in_=d2[:])
            t_w3 = cpool.tile([128, 256], BF16)
            nc.sync.dma_start(out=t_w3[:], in_=w3[:])
            # d3 [256,1] stored as [128, 2]: col0 = d3[:128], col1 = d3[128:]
            t_d3 = cpool.tile([128, 2], F32)
            nc.sync.dma_start(out=t_d3[:, 0:1], in_=d3[0:128, :])
            nc.sync.dma_start(out=t_d3[:, 1:2], in_=d3[128:256, :])
            t_negd3b = cpool.tile([128, 1], F32)
            nc.vector.tensor_scalar_mul(out=t_negd3b[:], in0=t_d3[:, 1:2], scalar1=-1.0)
            # w4 [256, 256] stored as [128, 512]: cols 0:256 = w4[0:128],
            # cols 256:512 = w4[128:256]
            t_w4 = cpool.tile([128, 512], BF16)
            # w4 [256, 256] -> [128, 512]: rows 0:128 cols 0:256 = w4[0:128],
            # cols 256:512 = w4[128:256]
            nc.sync.dma_start(out=t_w4[:, 0:256], in_=w4[0:128, :])
            nc.sync.dma_start(out=t_w4[:, 256:512], in_=w4[128:256, :])

            for t in range(t_pc):
                xin = io.tile([10, TILE], BF16, tag="xin")
                nc.sync.dma_start(out=xin[:], in_=featsT[:, t * TILE:(t + 1) * TILE])
                t_gi = io.tile([128, G // 16], U16, tag="gi")
                nc.sync.dma_start(out=t_gi[:], in_=gidx[t])

                p_h1 = ps.tile([64, TILE], F32, tag="h1")
                nc.tensor.matmul(out=p_h1[:], lhsT=t_a1[:], rhs=xin[:],
                                 start=True, stop=True)
                x2 = io.tile([64, TILE], BF16, tag="x2")
                nc.scalar.activation(out=x2[:], in_=p_h1[:], func=RELU)

                p_h2 = ps.tile([128, TILE], F32, tag="h2")
                nc.tensor.matmul(out=p_h2[:], lhsT=t_w2[:], rhs=x2[:],
                                 start=True, stop=True)
                x3 = io.tile([128, TILE], BF16, tag="x3")
                nc.scalar.activation(out=x3[:], in_=p_h2[:], func=RELU,
                                     bias=t_d2[:, 0:1])

                p_h3a = ps.tile([128, TILE], F32, tag="h3a")
                nc.tensor.matmul(out=p_h3a[:], lhsT=t_w3[:, 0:128], rhs=x3[:],
                                 start=True, stop=True)
                p_h3b = ps.tile([128, TILE], F32, tag="h3b")
                nc.tensor.matmul(out=p_h3b[:], lhsT=t_w3[:, 128:256], rhs=x3[:],
                                 start=True, stop=True)
                x4a = io.tile([128, TILE], BF16, tag="x4a")
                nc.scalar.activation(out=x4a[:], in_=p_h3a[:], func=RELU,
                                     bias=t_d3[:, 0:1])
                x4b = io.tile([128, TILE], BF16, tag="x4b")
                if P4_VARIANT == "actb":
                    # all relus on ACT; no shift (b4 fold excludes d3b term)
                    nc.scalar.activation(out=x4b[:], in_=p_h3b[:], func=RELU,
                                         bias=t_d3[:, 1:2])
                else:
                    # x4b' = max(h3b, -d3b) (shift; fold d3b@W4b into b4)
                    nc.vector.scalar_tensor_tensor(
                        out=x4b[:], in0=p_h3b[:], scalar=0.0,
                        in1=t_negd3b[:, 0:1].to_broadcast([128, TILE]),
                        op0=mybir.AluOpType.add, op1=mybir.AluOpType.max)

                p_h4 = psA.tile([128, 2 * TILE], F32, tag="h4")
                nc.tensor.matmul(out=p_h4[:, 0:TILE], lhsT=t_w4[:, 0:128],
                                 rhs=x4a[:], start=True, stop=False,
                                 skip_group_check=True)
                nc.tensor.matmul(out=p_h4[:, 0:TILE], lhsT=t_w4[:, 256:384],
                                 rhs=x4b[:], start=False, stop=True,
                                 skip_group_check=True)
                nc.tensor.matmul(out=p_h4[:, TILE:2 * TILE], lhsT=t_w4[:, 128:256],
                                 rhs=x4a[:], start=True, stop=False,
                                 skip_group_check=True)
                nc.tensor.matmul(out=p_h4[:, TILE:2 * TILE], lhsT=t_w4[:, 384:512],
                                 rhs=x4b[:], start=False, stop=True,
                                 skip_group_check=True)

                binmax = io.tile([128, 2 * TILE // BIN], F32, tag="binmax")
                nc.vector.tensor_reduce(
                    out=binmax[:],
                    in_=p_h4[:].rearrange("p (b e) -> p b e", e=BIN),
                    axis=mybir.AxisListType.X, op=mybir.AluOpType.max)

                gat = io.tile([128, G], F32, tag="gat")
                nc.gpsimd.indirect_copy(out=gat[:], data=binmax[:], idxs=t_gi[:],
                                        i_know_ap_gather_is_preferred=True)
                vox = io.tile([128, 2 * W], F32, tag="vox")
                nc.vector.tensor_reduce(
                    out=vox[:], in_=gat[:].rearrange("p (w e) -> p w e", e=BIN),
                    axis=mybir.AxisListType.X, op=mybir.AluOpType.max)
                nc.sync.dma_start(out=out[:, t * 2 * W:(t + 1) * 2 * W], in_=vox[:])
    _legalize_waits(nc)
    return nc


# ---------------------------------------------------------------------------
# host folding math (fp64)
# ---------------------------------------------------------------------------
def _fold(inputs, prep):
    """P1 on host: BN0 fold + analytic BN1 stats -> A1aug."""
    feats = np.asarray(inputs["feats"], np.float32)
    N = feats.shape[0]
    g0 = np.asarray(inputs["g0"], np.float64); be0 = np.asarray(inputs["be0"], np.float64)
    W1 = np.asarray(inputs["W1"], np.float64); b1 = np.asarray(inputs["b1"], np.float64)
    g1 = np.asarray(inputs["g1"], np.float64); be1 = np.asarray(inputs["be1"], np.float64)

    f64 = feats.astype(np.float64)
    m0 = f64.mean(0)
    M2f = f64.T @ f64 / N
    v0 = np.diag(M2f) - m0 * m0
    a0 = g0 / np.sqrt(v0 + EPS)
    A1 = W1 * a0[:, None]
    d1 = (be0 - m0 * a0) @ W1 + b1
    m1 = m0 @ A1 + d1
    Cov = M2f - np.outer(m0, m0)
    v1 = np.einsum("ij,ik,kj->j", A1, Cov, A1)
    a1 = g1 / np.sqrt(v1 + EPS)
    A1p = A1 * a1[None, :]
    d1p = (d1 - m1) * a1 + be1
    a1aug = np.empty((10, 64), np.float32)
    a1aug[0:9] = A1p.astype(np.float32)
    a1aug[9] = d1p.astype(np.float32)
    return dict(N=N, a1aug=a1aug)


def _fold2(mom2, N, inputs):
    """host fold after P2: moments of x2 -> W2aug(d2) for BN2."""
    W2 = np.asarray(inputs["W2"], np.float64); b2 = np.asarray(inputs["b2"], np.float64)
    g2 = np.asarray(inputs["g2"], np.float64); be2 = np.asarray(inputs["be2"], np.float64)
    S2 = mom2[:, :64].astype(np.float64)
    s2 = mom2[:, 64].astype(np.float64)
    m2 = s2 / N
    Cov2 = S2 / N - np.outer(m2, m2)
    m2h = m2 @ W2 + b2
    v2h = np.einsum("ij,ik,kj->j", W2, Cov2, W2)
    a2 = g2 / np.sqrt(v2h + EPS)
    W2p = (W2 * a2[None, :]).astype(np.float32)
    d2p = ((b2 - m2h) * a2 + be2).astype(np.float32)
    return W2p, d2p


def _fold3(mom3, N, d2p, inputs):
    """host fold after P3: moments of x3' (= x3 - d2p... see note) -> BN3.

    P3 computes x3 = relu(h2 + d2p) directly (bias via ACT), masked; so
    mom3 are moments of the true x3. Derive BN3 fold.
    """
    W3 = np.asarray(inputs["W3"], np.float64); b3 = np.asarray(inputs["b3"], np.float64)
    g3 = np.asarray(inputs["g3"], np.float64); be3 = np.asarray(inputs["be3"], np.float64)
    S3 = mom3[:, :128].astype(np.float64)
    s3 = mom3[:, 128].astype(np.float64)
    m3 = s3 / N
    Cov3 = S3 / N - np.outer(m3, m3)
    m3h = m3 @ W3 + b3
    v3h = np.einsum("ij,ik,kj->j", W3, Cov3, W3)
    a3 = g3 / np.sqrt(v3h + EPS)
    W3p = (W3 * a3[None, :]).astype(np.float32)
    d3p = ((b3 - m3h) * a3 + be3).astype(np.float32)
    return W3p, d3p


# ---------------------------------------------------------------------------
# main entry
# ---------------------------------------------------------------------------
import os as _os
import time as _time

LAST_STATS = {}


def _run_spmd(tag, nc, in_maps, core_ids):
    # NTFF profiling is unavailable under this axon client (no
    # antenv.axon_hooks), so when BASS_KERNEL_TRACE is set we also time a
    # second, jit-cached execution as the device-time proxy (includes axon
    # RPC + host<->device transfer, so it upper-bounds the NEFF time).
    t0 = _time.time()
    res = run_bass_kernel_spmd(nc, in_maps, core_ids=core_ids)
    wall1 = _time.time() - t0
    exec_wall2 = None
    if int(_os.environ.get("BASS_KERNEL_TRACE", "0")):
        t0 = _time.time()
        run_bass_kernel_spmd(nc, in_maps, core_ids=core_ids)
        exec_wall2 = _time.time() - t0
    LAST_STATS[tag] = dict(wall_s=wall1,
                           exec_ns=getattr(res, "exec_time_ns", None),
                           exec_wall2_s=exec_wall2)
    return res


def kernel(**inputs):
    feats = np.ascontiguousarray(np.asarray(inputs["feats"], np.float32))
    coords = np.ascontiguousarray(np.asarray(inputs["coords"], np.int32))
    num_voxels = int(np.asarray(inputs["num_voxels"]))
    prep = _prepare(feats, coords)
    V = prep["V"]
    assert V == num_voxels
    t_pc = prep["t_pc"]
    fold1 = _fold(inputs, prep)
    N = fold1["N"]
    a1aug = fold1["a1aug"]
    slabs = _feats_slabs(feats, prep)
    validc = prep["validc"].reshape(N_CORES, t_pc, 128, 4)
    gidx = prep["gidx"].reshape(N_CORES, t_pc, 128, G // 16)

    core_ids = list(range(N_CORES))

    # ---- P2
    nc2 = _build_p2(t_pc)
    in2 = [{"featsT": slabs[c], "validc": np.ascontiguousarray(validc[c]),
            "a1aug": a1aug} for c in core_ids]
    res2 = _run_spmd("p2", nc2, in2, core_ids)
    mom2 = np.sum([r["mom"].astype(np.float64).sum(0) for r in res2.results], axis=0)
    W2p, d2p = _fold2(mom2, N, inputs)

    # ---- P3
    nc3 = _build_p3(t_pc)
    d2col = np.ascontiguousarray(d2p.reshape(128, 1))
    in3 = [{"featsT": slabs[c], "validc": np.ascontiguousarray(validc[c]),
            "a1aug": a1aug, "w2": W2p, "d2": d2col} for c in core_ids]
    res3 = _run_spmd("p3", nc3, in3, core_ids)
    mom3 = np.sum([r["mom"].astype(np.float64).sum(0) for r in res3.results], axis=0)
    W3p, d3p = _fold3(mom3, N, d2p, inputs)

    # ---- P4
    W4 = np.asarray(inputs["W4"], np.float32)
    b4 = np.asarray(inputs["b4"], np.float64)
    # x4b' = max(h3b, -d3b) = x4b - d3b  =>  fold d3b @ W4[128:] into bias
    b4_eff = (b4 + d3p[128:].astype(np.float64) @ W4[128:256].astype(np.float64))
    nc4 = _build_p4(t_pc)
    d3col = np.ascontiguousarray(d3p.reshape(256, 1))
    import ml_dtypes
    bf = ml_dtypes.bfloat16
    in4 = [{"featsT": np.ascontiguousarray(slabs[c].astype(bf)),
            "gidx": np.ascontiguousarray(gidx[c]),
            "a1aug": a1aug.astype(bf), "w2": W2p.astype(bf), "d2": d2col,
            "w3": np.ascontiguousarray(W3p.astype(bf)), "d3": d3col,
            "w4": np.ascontiguousarray(W4.astype(bf))} for c in core_ids]
    res4 = _run_spmd("p4", nc4, in4, core_ids)

    # ---- assemble: rows follow jnp.unique(sorted occupied keys) order,
    # chunk slots of the same voxel max-combined; padding rows are empty
    # segments (-inf) with fill_value=0 coords.
    full = np.concatenate([r["out"] for r in res4.results], axis=1)  # [128, nt_pad*2W]
    nt = prep["nt"]
    kc = prep["kcount"][:nt]
    tt, ww = np.nonzero(np.arange(W)[None, :] < kc[:, None])
    vids = prep["vox_of"][tt, ww]
    colA = tt * 2 * W + ww
    colB = colA + W
    occ = prep["occ"]
    # vids follows occ order (chunks emitted in sorted-voxel order), so
    # same-voxel chunk slots are contiguous: max-combine via reduceat.
    assert np.all(np.diff(vids) >= 0)
    grp = np.flatnonzero(np.concatenate([[True], vids[1:] != vids[:-1]]))
    assert len(grp) == len(occ)
    pooled = np.full((V, 256), -np.inf, np.float32)
    pooled[:len(occ), 0:128] = np.maximum.reduceat(full[:, colA].T, grp, axis=0)
    pooled[:len(occ), 128:256] = np.maximum.reduceat(full[:, colB].T, grp, axis=0)
    pooled[:len(occ)] += b4_eff[None, :].astype(np.float32)

    unq_coords = np.zeros((V, 3), np.int64)
    unq_coords[:len(occ), 0] = occ // (GRID * GRID)
    unq_coords[:len(occ), 1] = (occ // GRID) % GRID
    unq_coords[:len(occ), 2] = occ % GRID
    return pooled.astype(np.float32), unq_coords
